# revision 4
# baseline (speedup 1.0000x reference)
"""Trainium2 Bass kernel for nn_AbstractFullyConnected (interval-bound MLP).

Math (per layer): x' = W@x + b;  box propagation in center/radius form:
  c' = W@c + b,  r' = |W|@r   (low = c-r, high = c+r)
followed by DeepPoly ReLU-box between layers.

Sharding: weight rows (output neurons) split across 8 cores; the three
matvecs per layer run on the PE with the small vectors as the stationary
operand and (host-pre-transposed, partition-folded) weight tiles as the
moving operand.  Between layers the [3, rows/core] result chunk is
AllGathered, PE-transposed back into partition-major ("folded") vector
layout, and bias + relu-box applied redundantly on every core.
"""

import os
import sys

os.environ.setdefault("MYCRO_LOCAL_CACHE", "1")
if "/opt/trn_rl_repo" not in sys.path:
    sys.path.insert(0, "/opt/trn_rl_repo")

import numpy as np

MEAN = 0.1307
SIGMA = 0.3081
EPS = 1e-07
P = 128
NCORES = 8


def build_bass(kin, hid, nout, ncores, sh_chunk_ktiles=4):
    """Build + compile the SPMD Bass program (identical on every core).

    kin: padded input dim (multiple of 128); hid: hidden dim; nout: output dim.
    DRAM inputs (per core):
      v1   [P, 3, T1]      folded (x, c, r) input vectors
      w1   [P, T1*SH]      folded W1^T row-shard  (col = t*SH + m)
      w2   [P, TH*SH]      folded W2^T row-shard
      w3   [P, TH*SH]      folded W3^T row-shard
      b1f/b2f/b3f [P, TH]  folded full biases
      w4   [P, TH*nout]    folded full W4^T
      w4a  [P, TH*nout]    folded full |W4|^T
      b4f  [3, nout]       rows (b4, b4, b4)
    Output: out [3, nout] = (x, low, high), identical on every core.
    """
    from contextlib import ExitStack
    from concourse import bacc, tile, mybir, masks

    f32 = mybir.dt.float32
    A = mybir.AluOpType
    AF = mybir.ActivationFunctionType

    T1 = kin // P
    TH = hid // P
    SH = hid // ncores
    NT = SH // P
    NCJ = 3 * ncores  # rows of the all-gathered chunk matrix

    nc = bacc.Bacc(
        "TRN2", target_bir_lowering=False, debug=False, num_devices=ncores
    )

    v1_d = nc.dram_tensor("v1", [P, 3, T1], f32, kind="ExternalInput")
    w1_d = nc.dram_tensor("w1", [P, T1 * SH], f32, kind="ExternalInput")
    w2_d = nc.dram_tensor("w2", [P, TH * SH], f32, kind="ExternalInput")
    w3_d = nc.dram_tensor("w3", [P, TH * SH], f32, kind="ExternalInput")
    b1f_d = nc.dram_tensor("b1f", [P, TH], f32, kind="ExternalInput")
    b2f_d = nc.dram_tensor("b2f", [P, TH], f32, kind="ExternalInput")
    b3f_d = nc.dram_tensor("b3f", [P, TH], f32, kind="ExternalInput")
    w4_d = nc.dram_tensor("w4", [P, TH * nout], f32, kind="ExternalInput")
    w4a_d = nc.dram_tensor("w4a", [P, TH * nout], f32, kind="ExternalInput")
    b4f_d = nc.dram_tensor("b4f", [3, nout], f32, kind="ExternalInput")
    out_d = nc.dram_tensor("out", [3, nout], f32, kind="ExternalOutput")

    rg = [list(range(ncores))]

    with tile.TileContext(nc) as tc, ExitStack() as ctx:
        const = ctx.enter_context(tc.tile_pool(name="const", bufs=1))
        wpool = ctx.enter_context(tc.tile_pool(name="wpool", bufs=1))
        apool = ctx.enter_context(tc.tile_pool(name="apool", bufs=6))
        vpool = ctx.enter_context(tc.tile_pool(name="vpool", bufs=1))
        tmp = ctx.enter_context(tc.tile_pool(name="tmp", bufs=1))
        pacc = ctx.enter_context(tc.tile_pool(name="pacc", bufs=2, space="PSUM"))
        pfold = ctx.enter_context(tc.tile_pool(name="pfold", bufs=4, space="PSUM"))
        dpool = ctx.enter_context(tc.tile_pool(name="dram", bufs=1, space="DRAM"))

        # ---- small constant inputs
        v1_sb = const.tile([P, 3, T1], f32, name="v1_sb")
        nc.sync.dma_start(out=v1_sb[:], in_=v1_d.ap())
        bf_sb = []
        for i, bd in enumerate((b1f_d, b2f_d, b3f_d)):
            t = const.tile([P, TH], f32, name=f"bf{i}_sb")
            nc.sync.dma_start(out=t[:], in_=bd.ap())
            bf_sb.append(t)
        w4_sb = const.tile([P, TH * nout], f32, name="w4_sb")
        nc.sync.dma_start(out=w4_sb[:], in_=w4_d.ap())
        w4a_sb = const.tile([P, TH * nout], f32, name="w4a_sb")
        nc.sync.dma_start(out=w4a_sb[:], in_=w4a_d.ap())
        b4f_sb = const.tile([3, nout], f32, name="b4f_sb")
        nc.sync.dma_start(out=b4f_sb[:], in_=b4f_d.ap())
        ident = const.tile([NCJ, NCJ], f32, name="ident")
        masks.make_identity(nc, ident[:])
        zeros = const.tile([P, TH], f32, name="zeros")
        nc.vector.memset(zeros[:], 0.0)

        # ---- weight shard DMAs (whole shard resident; chunked for overlap)
        def load_w(wd, T, lname):
            tiles = []
            for g0 in range(0, T, sh_chunk_ktiles):
                g1 = min(T, g0 + sh_chunk_ktiles)
                w = wpool.tile([P, (g1 - g0) * SH], f32, name=f"{lname}c{g0}")
                nc.sync.dma_start(out=w[:], in_=wd.ap()[:, g0 * SH : g1 * SH])
                for t in range(g0, g1):
                    tiles.append((w, (t - g0) * SH))
            return tiles

        w1_t = load_w(w1_d, T1, "w1")
        w2_t = load_w(w2_d, TH, "w2")
        w3_t = load_w(w3_d, TH, "w3")

        def layer(V, w_tiles, T, li):
            # PA[0:2] accumulates (W@x, W@c); PB row 2 accumulates |W|@r
            # (PB rows 0,1 are free byproducts |W|@x, |W|@c — the PE cost is
            # the rhs stream, independent of lhsT column count).
            PA = pacc.tile([2, SH], f32, name=f"pa{li}", tag="pa")
            PB = pacc.tile([3, SH], f32, name=f"pb{li}", tag="pb")
            for t in range(T):
                wt, off = w_tiles[t]
                rhs = wt[:, off : off + SH]
                nc.tensor.matmul(
                    PA[:], V[:, 0:2, t], rhs, start=(t == 0), stop=(t == T - 1)
                )
                ab = apool.tile([P, SH], f32, name=f"ab{li}_{t}", tag="abs")
                if t % 2 == 0:
                    nc.scalar.activation(ab[:], rhs, AF.Abs)
                else:
                    # abs via sign-bit clear (fp32 bit pattern & 0x7fffffff)
                    i32 = mybir.dt.int32
                    nc.vector.tensor_scalar(
                        ab[:].bitcast(i32),
                        rhs.bitcast(i32),
                        0x7FFFFFFF,
                        None,
                        A.bitwise_and,
                    )
                nc.tensor.matmul(
                    PB[:], V[:, 0:3, t], ab[:], start=(t == 0), stop=(t == T - 1)
                )
            return PA, PB

        def boundary(PA, PB, bi):
            # AllGather the raw [3, SH] chunk, fold back to partition-major,
            # then bias + x-relu + relu-box (redundantly on every core).
            pas = tmp.tile([2, SH], f32, name=f"pas{bi}")
            pbs = tmp.tile([3, SH], f32, name=f"pbs{bi}")
            nc.vector.tensor_copy(pas[:], PA[:])
            nc.vector.tensor_copy(pbs[:], PB[:])
            agi = dpool.tile([3, SH], f32, name=f"agi{bi}")
            ago = dpool.tile([NCJ, SH], f32, name=f"ago{bi}", addr_space="Shared")
            nc.sync.dma_start(out=agi[0:2, :], in_=pas[:])
            nc.sync.dma_start(out=agi[2:3, :], in_=pbs[2:3, :])
            nc.gpsimd.collective_compute(
                "AllGather",
                A.bypass,
                replica_groups=rg,
                ins=[agi.opt()],
                outs=[ago.opt()],
            )
            G = tmp.tile([NCJ, SH], f32, name=f"g{bi}")
            nc.sync.dma_start(out=G[:], in_=ago[:])
            V = vpool.tile([P, 3, TH], f32, name=f"v{bi + 2}")
            for tp in range(NT):
                pf = pfold.tile([P, NCJ], f32, name=f"pf{bi}_{tp}", tag="pf")
                nc.tensor.transpose(pf[:], G[:, tp * P : (tp + 1) * P], ident[:])
                src = pf.rearrange("p (c j) -> p j c", j=3)
                nc.vector.tensor_copy(V[:, :, tp::NT], src)
            xs = V[:, 0, :]
            cs = V[:, 1, :]
            rs = V[:, 2, :]
            bf = bf_sb[bi][:]
            nc.vector.tensor_add(xs, xs, bf)
            nc.vector.tensor_add(cs, cs, bf)
            nc.vector.tensor_scalar(xs, xs, 0.0, None, A.max)
            low = tmp.tile([P, TH], f32, name=f"low{bi}")
            high = tmp.tile([P, TH], f32, name=f"high{bi}")
            nc.vector.tensor_sub(low[:], cs, rs)
            nc.vector.tensor_add(high[:], cs, rs)
            # crossing upper bound: h' = high^2/(high-low+EPS) - low*high/(high-low)
            d1 = tmp.tile([P, TH], f32, name=f"d1_{bi}")
            nc.vector.scalar_tensor_tensor(
                d1[:], high[:], EPS, low[:], A.add, A.subtract
            )
            d2 = tmp.tile([P, TH], f32, name=f"d2_{bi}")
            nc.vector.tensor_sub(d2[:], high[:], low[:])
            r1 = tmp.tile([P, TH], f32, name=f"r1_{bi}")
            nc.vector.reciprocal(r1[:], d1[:])
            r2 = tmp.tile([P, TH], f32, name=f"r2_{bi}")
            nc.vector.reciprocal(r2[:], d2[:])
            t1 = tmp.tile([P, TH], f32, name=f"t1_{bi}")
            nc.vector.tensor_mul(t1[:], high[:], r1[:])
            t2 = tmp.tile([P, TH], f32, name=f"t2_{bi}")
            nc.vector.tensor_mul(t2[:], low[:], r2[:])
            t3 = tmp.tile([P, TH], f32, name=f"t3_{bi}")
            nc.vector.tensor_sub(t3[:], t1[:], t2[:])
            hp2 = tmp.tile([P, TH], f32, name=f"hp2_{bi}")
            nc.vector.scalar_tensor_tensor(
                hp2[:], t3[:], 0.5, high[:], A.mult, A.mult
            )
            u8 = mybir.dt.uint8
            ml = tmp.tile([P, TH], u8, name=f"ml_{bi}")
            nc.vector.tensor_scalar(ml[:], low[:], 0.0, None, A.is_lt)
            mh = tmp.tile([P, TH], u8, name=f"mh_{bi}")
            nc.vector.tensor_scalar(mh[:], high[:], 0.0, None, A.is_gt)
            mc = tmp.tile([P, TH], u8, name=f"mc_{bi}")
            nc.vector.tensor_mul(mc[:], ml[:], mh[:])
            nc.vector.copy_predicated(cs, mc[:], hp2[:])
            nc.vector.copy_predicated(rs, mc[:], hp2[:])
            md = tmp.tile([P, TH], u8, name=f"md_{bi}")
            nc.vector.tensor_scalar(md[:], high[:], 0.0, None, A.is_le)
            nc.vector.copy_predicated(cs, md[:], zeros[:])
            nc.vector.copy_predicated(rs, md[:], zeros[:])
            return V

        PA, PB = layer(v1_sb, w1_t, T1, 1)
        V2 = boundary(PA, PB, 0)
        PA, PB = layer(V2, w2_t, TH, 2)
        V3 = boundary(PA, PB, 1)
        PA, PB = layer(V3, w3_t, TH, 3)
        V4 = boundary(PA, PB, 2)

        # ---- final layer, computed in full on every core:
        # pass 1 lhsT (x, c, c) over W4^T; pass 2 lhsT (0, -r, +r) over |W4|^T
        # accumulate into one PSUM → rows (x, low, high) before bias.
        VX = vpool.tile([P, 3, TH], f32, name="vx4")
        VR = vpool.tile([P, 3, TH], f32, name="vr4")
        nc.vector.tensor_copy(VX[:, 0:2, :], V4[:, 0:2, :])
        nc.vector.tensor_copy(VX[:, 2, :], V4[:, 1, :])
        nc.vector.memset(VR[:, 0, :], 0.0)
        nc.vector.tensor_scalar(VR[:, 1, :], V4[:, 2, :], -1.0, None, A.mult)
        nc.vector.tensor_copy(VR[:, 2, :], V4[:, 2, :])
        PO = pacc.tile([3, nout], f32, name="po", tag="pa")
        for t in range(TH):
            nc.tensor.matmul(
                PO[:],
                VX[:, :, t],
                w4_sb[:, t * nout : (t + 1) * nout],
                start=(t == 0),
                stop=False,
            )
            nc.tensor.matmul(
                PO[:],
                VR[:, :, t],
                w4a_sb[:, t * nout : (t + 1) * nout],
                start=False,
                stop=(t == TH - 1),
            )
        res = tmp.tile([3, nout], f32, name="res")
        nc.vector.tensor_copy(res[:], PO[:])
        nc.vector.tensor_add(res[:], res[:], b4f_sb[:])
        nc.sync.dma_start(out=out_d.ap(), in_=res[:])

    nc.compile()
    return nc


def prepare_in_maps(x, low, high, W1, b1, W2, b2, W3, b3, W4, b4, ncores=NCORES):
    """Host-side sharding/layout prep. Returns (in_maps, kin, hid, nout)."""
    f = np.float32
    x = np.asarray(x, f).reshape(-1)
    lo = np.asarray(low, f).reshape(-1)
    hi = np.asarray(high, f).reshape(-1)
    W1 = np.asarray(W1, f)
    W2 = np.asarray(W2, f)
    W3 = np.asarray(W3, f)
    W4 = np.asarray(W4, f)
    b1 = np.asarray(b1, f)
    b2 = np.asarray(b2, f)
    b3 = np.asarray(b3, f)
    b4 = np.asarray(b4, f)

    hid, kin_raw = W1.shape
    nout = W4.shape[0]
    kin = ((kin_raw + P - 1) // P) * P
    pad = kin - kin_raw
    SH = hid // ncores
    TH = hid // P

    mean = f(MEAN)
    sigma = f(SIGMA)
    xn = (x - mean) / sigma
    ln = (lo - mean) / sigma
    hn = (hi - mean) / sigma
    c = (ln + hn) * f(0.5)
    r = (hn - ln) * f(0.5)

    def padv(v):
        return np.pad(v, (0, pad)).astype(f)

    v1 = np.stack([padv(xn), padv(c), padv(r)])  # [3, kin]
    v1 = np.ascontiguousarray(v1.reshape(3, kin // P, P).transpose(2, 0, 1), f)

    def fold_w(wt, T):
        # wt [K, M_cols] (K = T*128) -> [P, T*M_cols], col = t*M + m
        M = wt.shape[1]
        return np.ascontiguousarray(
            wt.reshape(T, P, M).transpose(1, 0, 2).reshape(P, T * M), f
        )

    def fold_b(b):
        return np.ascontiguousarray(b.reshape(TH, P).T, f)

    W1p = np.pad(W1, ((0, 0), (0, pad)))  # [hid, kin]
    w4t = np.ascontiguousarray(W4.T)  # [hid, nout]
    w4f = fold_w(w4t, TH)
    w4af = fold_w(np.abs(w4t), TH)
    b4f3 = np.ascontiguousarray(np.stack([b4, b4, b4]), f)
    b1f = fold_b(b1)
    b2f = fold_b(b2)
    b3f = fold_b(b3)

    in_maps = []
    for cidx in range(ncores):
        sl = slice(cidx * SH, (cidx + 1) * SH)
        in_maps.append(
            {
                "v1": v1,
                "w1": fold_w(np.ascontiguousarray(W1p[sl].T), kin // P),
                "w2": fold_w(np.ascontiguousarray(W2[sl].T), TH),
                "w3": fold_w(np.ascontiguousarray(W3[sl].T), TH),
                "b1f": b1f,
                "b2f": b2f,
                "b3f": b3f,
                "w4": w4f,
                "w4a": w4af,
                "b4f": b4f3,
            }
        )
    return in_maps, kin, hid, nout


_BUILT = {}


def _get_program(kin, hid, nout, ncores):
    key = (kin, hid, nout, ncores)
    if key not in _BUILT:
        _BUILT[key] = build_bass(kin, hid, nout, ncores)
    return _BUILT[key]


def kernel(x, low, high, W1, b1, W2, b2, W3, b3, W4, b4):
    from concourse.bass_utils import run_bass_kernel_spmd

    in_maps, kin, hid, nout = prepare_in_maps(
        x, low, high, W1, b1, W2, b2, W3, b3, W4, b4, NCORES
    )
    prog = _get_program(kin, hid, nout, NCORES)
    res = run_bass_kernel_spmd(prog, in_maps, list(range(NCORES)))
    o = np.asarray(res.results[0]["out"], np.float32)
    return (o[0].copy(), o[1].copy(), o[2].copy())


if __name__ == "__main__":
    # smoke test with random data at full size
    rng = np.random.default_rng(0)
    hid, kin, nout = 4096, 784, 10
    x = rng.random((1, 1, 28, 28), dtype=np.float32)
    args = dict(
        x=x,
        low=x - 0.05,
        high=x + 0.05,
        W1=rng.standard_normal((hid, kin), dtype=np.float32) / np.sqrt(kin),
        b1=rng.standard_normal(hid, dtype=np.float32) * 0.01,
        W2=rng.standard_normal((hid, hid), dtype=np.float32) / np.sqrt(hid),
        b2=rng.standard_normal(hid, dtype=np.float32) * 0.01,
        W3=rng.standard_normal((hid, hid), dtype=np.float32) / np.sqrt(hid),
        b3=rng.standard_normal(hid, dtype=np.float32) * 0.01,
        W4=rng.standard_normal((nout, hid), dtype=np.float32) / np.sqrt(hid),
        b4=rng.standard_normal(nout, dtype=np.float32) * 0.01,
    )
    out = kernel(**args)
    print([np.asarray(o) for o in out])


# revision 9
# speedup vs baseline: 828.1321x; 828.1321x over previous
"""Trainium2 Bass kernel for nn_AbstractFullyConnected (interval-bound MLP).

Math (per layer): x' = W@x + b;  box propagation in center/radius form:
  c' = W@c + b,  r' = |W|@r   (low = c-r, high = c+r)
followed by DeepPoly ReLU-box between layers.

Sharding: weight rows (output neurons) split across 8 cores; the three
matvecs per layer run on the PE with the small vectors as the stationary
operand and (host-pre-transposed, partition-folded) weight tiles as the
moving operand.  Between layers the [3, rows/core] result chunk is
AllGathered, PE-transposed back into partition-major ("folded") vector
layout, and bias + relu-box applied redundantly on every core.
"""

import os
import sys

os.environ.setdefault("MYCRO_LOCAL_CACHE", "1")
if "/opt/trn_rl_repo" not in sys.path:
    sys.path.insert(0, "/opt/trn_rl_repo")

import numpy as np

MEAN = 0.1307
SIGMA = 0.3081
EPS = 1e-07
P = 128
NCORES = 8


def build_bass(kin, hid, nout, ncores, sh_chunk_ktiles=4, repeat=1):
    """Build + compile the SPMD Bass program (identical on every core).

    kin: padded input dim (multiple of 128); hid: hidden dim; nout: output dim.
    DRAM inputs (per core):
      v1   [P, 3, T1]      folded (x, c, r) input vectors
      w1   [P, T1*SH]      folded W1^T row-shard  (col = t*SH + m)
      w2   [P, TH*SH]      folded W2^T row-shard
      w3   [P, TH*SH]      folded W3^T row-shard
      b1f/b2f/b3f [P, TH]  folded full biases
      w4   [P, TH*nout]    folded full W4^T
      w4a  [P, TH*nout]    folded full |W4|^T
      b4f  [3, nout]       rows (b4, b4, b4)
    Output: out [3, nout] = (x, low, high), identical on every core.
    """
    from contextlib import ExitStack
    from concourse import bacc, tile, mybir, masks

    f32 = mybir.dt.float32
    A = mybir.AluOpType
    AF = mybir.ActivationFunctionType

    T1 = kin // P
    TH = hid // P
    SH = hid // ncores
    NT = SH // P
    NCJ = 3 * ncores  # rows of the all-gathered chunk matrix

    nc = bacc.Bacc(
        "TRN2", target_bir_lowering=False, debug=False, num_devices=ncores
    )

    v1_d = nc.dram_tensor("v1", [P, 3, T1], f32, kind="ExternalInput")
    w1_d = nc.dram_tensor("w1", [P, T1 * SH], f32, kind="ExternalInput")
    w2_d = nc.dram_tensor("w2", [P, TH * SH], f32, kind="ExternalInput")
    w3_d = nc.dram_tensor("w3", [P, TH * SH], f32, kind="ExternalInput")
    b1f_d = nc.dram_tensor("b1f", [P, TH], f32, kind="ExternalInput")
    b2f_d = nc.dram_tensor("b2f", [P, TH], f32, kind="ExternalInput")
    b3f_d = nc.dram_tensor("b3f", [P, TH], f32, kind="ExternalInput")
    w4_d = nc.dram_tensor("w4", [P, TH * nout], f32, kind="ExternalInput")
    w4a_d = nc.dram_tensor("w4a", [P, TH * nout], f32, kind="ExternalInput")
    b4f_d = nc.dram_tensor("b4f", [3, nout], f32, kind="ExternalInput")
    out_d = nc.dram_tensor("out", [3, nout], f32, kind="ExternalOutput")

    rg = [list(range(ncores))]

    with tile.TileContext(nc) as tc, ExitStack() as ctx:
        const = ctx.enter_context(tc.tile_pool(name="const", bufs=1))
        wpool = ctx.enter_context(tc.tile_pool(name="wpool", bufs=1))
        apool = ctx.enter_context(tc.tile_pool(name="apool", bufs=6))
        vpool = ctx.enter_context(tc.tile_pool(name="vpool", bufs=1))
        tmp = ctx.enter_context(tc.tile_pool(name="tmp", bufs=1))
        pacc = ctx.enter_context(tc.tile_pool(name="pacc", bufs=2, space="PSUM"))
        pfold = ctx.enter_context(tc.tile_pool(name="pfold", bufs=4, space="PSUM"))
        dpool = ctx.enter_context(tc.tile_pool(name="dram", bufs=2, space="DRAM"))

        # ---- small constant inputs
        v1_sb = const.tile([P, 3, T1], f32, name="v1_sb")
        nc.sync.dma_start(out=v1_sb[:], in_=v1_d.ap())
        bf_sb = []
        for i, bd in enumerate((b1f_d, b2f_d, b3f_d)):
            t = const.tile([P, TH], f32, name=f"bf{i}_sb")
            nc.sync.dma_start(out=t[:], in_=bd.ap())
            bf_sb.append(t)
        w4_sb = const.tile([P, TH * nout], f32, name="w4_sb")
        nc.sync.dma_start(out=w4_sb[:], in_=w4_d.ap())
        w4a_sb = const.tile([P, TH * nout], f32, name="w4a_sb")
        nc.sync.dma_start(out=w4a_sb[:], in_=w4a_d.ap())
        b4f_sb = const.tile([3, nout], f32, name="b4f_sb")
        nc.sync.dma_start(out=b4f_sb[:], in_=b4f_d.ap())
        ident = const.tile([NCJ, NCJ], f32, name="ident")
        masks.make_identity(nc, ident[:])
        zeros = const.tile([P, TH], f32, name="zeros")
        nc.vector.memset(zeros[:], 0.0)

        # ---- weight shard DMAs (whole shard resident; chunked for overlap)
        def load_w(wd, T, lname, it):
            tiles = []
            for g0 in range(0, T, sh_chunk_ktiles):
                g1 = min(T, g0 + sh_chunk_ktiles)
                w = wpool.tile(
                    [P, (g1 - g0) * SH],
                    f32,
                    name=f"{lname}c{g0}_{it}",
                    tag=f"{lname}c{g0}",
                )
                nc.sync.dma_start(out=w[:], in_=wd.ap()[:, g0 * SH : g1 * SH])
                for t in range(g0, g1):
                    tiles.append((w, (t - g0) * SH))
            return tiles

        def layer(V, w_tiles, T, li):
            # PA[0:2] accumulates (W@x, W@c); PB row 2 accumulates |W|@r
            # (PB rows 0,1 are free byproducts |W|@x, |W|@c — the PE cost is
            # the rhs stream, independent of lhsT column count).
            PA = pacc.tile([2, SH], f32, name=f"pa{li}", tag="pa")
            PB = pacc.tile([3, SH], f32, name=f"pb{li}", tag="pb")
            for t in range(T):
                wt, off = w_tiles[t]
                rhs = wt[:, off : off + SH]
                nc.tensor.matmul(
                    PA[:], V[:, 0:2, t], rhs, start=(t == 0), stop=(t == T - 1)
                )
                ab = apool.tile([P, SH], f32, name=f"ab{li}_{t}", tag="abs")
                if t % 2 == 0:
                    nc.scalar.activation(ab[:], rhs, AF.Abs)
                else:
                    # abs via sign-bit clear (fp32 bit pattern & 0x7fffffff)
                    i32 = mybir.dt.int32
                    nc.vector.tensor_scalar(
                        ab[:].bitcast(i32),
                        rhs.bitcast(i32),
                        0x7FFFFFFF,
                        None,
                        A.bitwise_and,
                    )
                nc.tensor.matmul(
                    PB[:], V[:, 0:3, t], ab[:], start=(t == 0), stop=(t == T - 1)
                )
            return PA, PB

        def boundary(PA, PB, bi):
            # AllGather the raw [3, SH] chunk, fold back to partition-major,
            # then bias + x-relu + relu-box (redundantly on every core).
            pas = tmp.tile([2, SH], f32, name=f"pas{bi}")
            pbs = tmp.tile([3, SH], f32, name=f"pbs{bi}")
            nc.vector.tensor_copy(pas[:], PA[:])
            nc.vector.tensor_copy(pbs[:], PB[:])
            agi = dpool.tile([3, SH], f32, name=f"agi{bi}")
            ago = dpool.tile([NCJ, SH], f32, name=f"ago{bi}", addr_space="Shared")
            nc.sync.dma_start(out=agi[0:2, :], in_=pas[:])
            nc.sync.dma_start(out=agi[2:3, :], in_=pbs[2:3, :])
            nc.gpsimd.collective_compute(
                "AllGather",
                A.bypass,
                replica_groups=rg,
                ins=[agi.opt()],
                outs=[ago.opt()],
            )
            G = tmp.tile([NCJ, SH], f32, name=f"g{bi}")
            nc.sync.dma_start(out=G[:], in_=ago[:])
            V = vpool.tile([P, 3, TH], f32, name=f"v{bi + 2}")
            for tp in range(NT):
                pf = pfold.tile([P, NCJ], f32, name=f"pf{bi}_{tp}", tag="pf")
                nc.tensor.transpose(pf[:], G[:, tp * P : (tp + 1) * P], ident[:])
                src = pf.rearrange("p (c j) -> p j c", j=3)
                nc.vector.tensor_copy(V[:, :, tp::NT], src)
            xs = V[:, 0, :]
            cs = V[:, 1, :]
            rs = V[:, 2, :]
            bf = bf_sb[bi][:]
            nc.vector.tensor_add(xs, xs, bf)
            nc.vector.tensor_add(cs, cs, bf)
            nc.vector.tensor_scalar(xs, xs, 0.0, None, A.max)
            low = tmp.tile([P, TH], f32, name=f"low{bi}")
            high = tmp.tile([P, TH], f32, name=f"high{bi}")
            nc.vector.tensor_sub(low[:], cs, rs)
            nc.vector.tensor_add(high[:], cs, rs)
            # crossing upper bound: h' = high^2/(high-low+EPS) - low*high/(high-low)
            d1 = tmp.tile([P, TH], f32, name=f"d1_{bi}")
            nc.vector.scalar_tensor_tensor(
                d1[:], high[:], EPS, low[:], A.add, A.subtract
            )
            d2 = tmp.tile([P, TH], f32, name=f"d2_{bi}")
            nc.vector.tensor_sub(d2[:], high[:], low[:])
            r1 = tmp.tile([P, TH], f32, name=f"r1_{bi}")
            nc.vector.reciprocal(r1[:], d1[:])
            r2 = tmp.tile([P, TH], f32, name=f"r2_{bi}")
            nc.vector.reciprocal(r2[:], d2[:])
            t1 = tmp.tile([P, TH], f32, name=f"t1_{bi}")
            nc.vector.tensor_mul(t1[:], high[:], r1[:])
            t2 = tmp.tile([P, TH], f32, name=f"t2_{bi}")
            nc.vector.tensor_mul(t2[:], low[:], r2[:])
            t3 = tmp.tile([P, TH], f32, name=f"t3_{bi}")
            nc.vector.tensor_sub(t3[:], t1[:], t2[:])
            hp2 = tmp.tile([P, TH], f32, name=f"hp2_{bi}")
            nc.vector.scalar_tensor_tensor(
                hp2[:], t3[:], 0.5, high[:], A.mult, A.mult
            )
            u8 = mybir.dt.uint8
            ml = tmp.tile([P, TH], u8, name=f"ml_{bi}")
            nc.vector.tensor_scalar(ml[:], low[:], 0.0, None, A.is_lt)
            mh = tmp.tile([P, TH], u8, name=f"mh_{bi}")
            nc.vector.tensor_scalar(mh[:], high[:], 0.0, None, A.is_gt)
            mc = tmp.tile([P, TH], u8, name=f"mc_{bi}")
            nc.vector.tensor_mul(mc[:], ml[:], mh[:])
            nc.vector.copy_predicated(cs, mc[:], hp2[:])
            nc.vector.copy_predicated(rs, mc[:], hp2[:])
            md = tmp.tile([P, TH], u8, name=f"md_{bi}")
            nc.vector.tensor_scalar(md[:], high[:], 0.0, None, A.is_le)
            nc.vector.copy_predicated(cs, md[:], zeros[:])
            nc.vector.copy_predicated(rs, md[:], zeros[:])
            return V

        def final(V4):
            # ---- final layer, computed in full on every core:
            # pass 1 lhsT (x, c, c) over W4^T; pass 2 lhsT (0, -r, +r) over
            # |W4|^T, accumulated into one PSUM → rows (x, low, high).
            VX = vpool.tile([P, 3, TH], f32, name="vx4")
            VR = vpool.tile([P, 3, TH], f32, name="vr4")
            nc.vector.tensor_copy(VX[:, 0:2, :], V4[:, 0:2, :])
            nc.vector.tensor_copy(VX[:, 2, :], V4[:, 1, :])
            nc.vector.memset(VR[:, 0, :], 0.0)
            nc.vector.tensor_scalar(VR[:, 1, :], V4[:, 2, :], -1.0, None, A.mult)
            nc.vector.tensor_copy(VR[:, 2, :], V4[:, 2, :])
            PO = pacc.tile([3, nout], f32, name="po", tag="pa")
            for t in range(TH):
                nc.tensor.matmul(
                    PO[:],
                    VX[:, :, t],
                    w4_sb[:, t * nout : (t + 1) * nout],
                    start=(t == 0),
                    stop=False,
                )
                nc.tensor.matmul(
                    PO[:],
                    VR[:, :, t],
                    w4a_sb[:, t * nout : (t + 1) * nout],
                    start=False,
                    stop=(t == TH - 1),
                )
            res = tmp.tile([3, nout], f32, name="res")
            nc.vector.tensor_copy(res[:], PO[:])
            nc.vector.tensor_add(res[:], res[:], b4f_sb[:])
            nc.sync.dma_start(out=out_d.ap(), in_=res[:])

        for it in range(repeat):
            w1_t = load_w(w1_d, T1, "w1", it)
            w2_t = load_w(w2_d, TH, "w2", it)
            w3_t = load_w(w3_d, TH, "w3", it)
            PA, PB = layer(v1_sb, w1_t, T1, 1)
            V2 = boundary(PA, PB, 0)
            PA, PB = layer(V2, w2_t, TH, 2)
            V3 = boundary(PA, PB, 1)
            PA, PB = layer(V3, w3_t, TH, 3)
            V4 = boundary(PA, PB, 2)
            final(V4)

    nc.compile()
    return nc


def prepare_in_maps(x, low, high, W1, b1, W2, b2, W3, b3, W4, b4, ncores=NCORES):
    """Host-side sharding/layout prep. Returns (in_maps, kin, hid, nout)."""
    f = np.float32
    x = np.asarray(x, f).reshape(-1)
    lo = np.asarray(low, f).reshape(-1)
    hi = np.asarray(high, f).reshape(-1)
    W1 = np.asarray(W1, f)
    W2 = np.asarray(W2, f)
    W3 = np.asarray(W3, f)
    W4 = np.asarray(W4, f)
    b1 = np.asarray(b1, f)
    b2 = np.asarray(b2, f)
    b3 = np.asarray(b3, f)
    b4 = np.asarray(b4, f)

    hid, kin_raw = W1.shape
    nout = W4.shape[0]
    kin = ((kin_raw + P - 1) // P) * P
    pad = kin - kin_raw
    SH = hid // ncores
    TH = hid // P

    mean = f(MEAN)
    sigma = f(SIGMA)
    xn = (x - mean) / sigma
    ln = (lo - mean) / sigma
    hn = (hi - mean) / sigma
    c = (ln + hn) * f(0.5)
    r = (hn - ln) * f(0.5)

    def padv(v):
        return np.pad(v, (0, pad)).astype(f)

    v1 = np.stack([padv(xn), padv(c), padv(r)])  # [3, kin]
    v1 = np.ascontiguousarray(v1.reshape(3, kin // P, P).transpose(2, 0, 1), f)

    def fold_w(wt, T):
        # wt [K, M_cols] (K = T*128) -> [P, T*M_cols], col = t*M + m
        M = wt.shape[1]
        return np.ascontiguousarray(
            wt.reshape(T, P, M).transpose(1, 0, 2).reshape(P, T * M), f
        )

    def fold_b(b):
        return np.ascontiguousarray(b.reshape(TH, P).T, f)

    W1p = np.pad(W1, ((0, 0), (0, pad)))  # [hid, kin]
    w4t = np.ascontiguousarray(W4.T)  # [hid, nout]
    w4f = fold_w(w4t, TH)
    w4af = fold_w(np.abs(w4t), TH)
    b4f3 = np.ascontiguousarray(np.stack([b4, b4, b4]), f)
    b1f = fold_b(b1)
    b2f = fold_b(b2)
    b3f = fold_b(b3)

    in_maps = []
    for cidx in range(ncores):
        sl = slice(cidx * SH, (cidx + 1) * SH)
        in_maps.append(
            {
                "v1": v1,
                "w1": fold_w(np.ascontiguousarray(W1p[sl].T), kin // P),
                "w2": fold_w(np.ascontiguousarray(W2[sl].T), TH),
                "w3": fold_w(np.ascontiguousarray(W3[sl].T), TH),
                "b1f": b1f,
                "b2f": b2f,
                "b3f": b3f,
                "w4": w4f,
                "w4a": w4af,
                "b4f": b4f3,
            }
        )
    return in_maps, kin, hid, nout


_BUILT = {}


def _get_program(kin, hid, nout, ncores):
    key = (kin, hid, nout, ncores)
    if key not in _BUILT:
        _BUILT[key] = build_bass(kin, hid, nout, ncores)
    return _BUILT[key]


def kernel(x, low, high, W1, b1, W2, b2, W3, b3, W4, b4):
    from concourse.bass_utils import run_bass_kernel_spmd

    in_maps, kin, hid, nout = prepare_in_maps(
        x, low, high, W1, b1, W2, b2, W3, b3, W4, b4, NCORES
    )
    prog = _get_program(kin, hid, nout, NCORES)
    res = run_bass_kernel_spmd(prog, in_maps, list(range(NCORES)))
    o = np.asarray(res.results[0]["out"], np.float32)
    return (o[0].copy(), o[1].copy(), o[2].copy())


if __name__ == "__main__":
    # smoke test with random data at full size
    rng = np.random.default_rng(0)
    hid, kin, nout = 4096, 784, 10
    x = rng.random((1, 1, 28, 28), dtype=np.float32)
    args = dict(
        x=x,
        low=x - 0.05,
        high=x + 0.05,
        W1=rng.standard_normal((hid, kin), dtype=np.float32) / np.sqrt(kin),
        b1=rng.standard_normal(hid, dtype=np.float32) * 0.01,
        W2=rng.standard_normal((hid, hid), dtype=np.float32) / np.sqrt(hid),
        b2=rng.standard_normal(hid, dtype=np.float32) * 0.01,
        W3=rng.standard_normal((hid, hid), dtype=np.float32) / np.sqrt(hid),
        b3=rng.standard_normal(hid, dtype=np.float32) * 0.01,
        W4=rng.standard_normal((nout, hid), dtype=np.float32) / np.sqrt(hid),
        b4=rng.standard_normal(nout, dtype=np.float32) * 0.01,
    )
    out = kernel(**args)
    print([np.asarray(o) for o in out])
